# revision 5
# baseline (speedup 1.0000x reference)
"""Causal self-attention (B=2, T=2048, D=1024, H=16, Dh=64) on 8 Trainium2 cores.

Sharding: (batch, head-group) — core c handles batch c//4 and heads 4*(c%4)..+4.
Each core computes Q/K/V projections for its 4 heads, causal attention, and a
partial output projection (its head-columns of Wo); the host sums the 4 partial
outputs per batch and adds bo.

Per-core layouts (all matmuls in float32r = fp32 bits on the fast PE path):
  xT   [D=1024, T=2048]   x[b] transposed (d on partitions)
  qT/kT [256, T]          head-major projections, d-of-head on partitions
  v    [T, 4, 65]         natural orientation + ones column (softmax denom trick)
  S^T  [j, i] chunks      scores transposed: exp() then causal zeroing, no max
                          subtraction (|scores| <= ~10, fp32-safe)
  y'^T [65, T] psum       unnormalized attention out + denominator row via the
                          ones column of V; normalized by PE-broadcast reciprocal
"""
import numpy as np

import concourse.bacc as bacc
import concourse.bass as bass
import concourse.mybir as mybir
import concourse.tile as tile
from concourse.bass_utils import run_bass_kernel_spmd

F32 = mybir.dt.float32
F32R = mybir.dt.float32r

B, T, D = 2, 2048, 1024
NH_LOC, DH = 4, 64          # heads per core, head dim
M = NH_LOC * DH             # 256 local qkv dims
KD = D // 128               # 8 contraction chunks
NT = T // 128               # 16 t-blocks
NC = T // 512               # 4 512-chunks
Exp = mybir.ActivationFunctionType.Exp


def _build():
    nc = bacc.Bacc("TRN2", target_bir_lowering=False, debug=False, num_devices=8)

    xT = nc.dram_tensor("xT", [D, T], F32R, kind="ExternalInput")
    wqT = nc.dram_tensor("wqT", [D, M], F32R, kind="ExternalInput")
    wkT = nc.dram_tensor("wkT", [D, M], F32R, kind="ExternalInput")
    wvT = nc.dram_tensor("wvT", [D, M], F32R, kind="ExternalInput")
    bq = nc.dram_tensor("bq", [M], F32, kind="ExternalInput")
    bk = nc.dram_tensor("bk", [M], F32, kind="ExternalInput")
    bv = nc.dram_tensor("bv", [M], F32R, kind="ExternalInput")
    woT = nc.dram_tensor("woT", [M, D], F32R, kind="ExternalInput")
    ones_d = nc.dram_tensor("ones_d", [128, 128], F32R, kind="ExternalInput")
    outp = nc.dram_tensor("outp", [T, D], F32, kind="ExternalOutput")

    with tile.TileContext(nc) as tc:
        with (
            tc.tile_pool(name="const", bufs=1) as const,
            tc.tile_pool(name="xpool", bufs=1) as xpool,
        ):
            xs = xpool.tile([128, KD, T], F32R)
            nc.sync.dma_start(out=xs, in_=xT.rearrange("(dd p) t -> p dd t", p=128))
            wq_s = const.tile([128, KD, M], F32R)
            nc.sync.dma_start(out=wq_s, in_=wqT.rearrange("(dd p) m -> p dd m", p=128))
            wk_s = const.tile([128, KD, M], F32R)
            nc.sync.dma_start(out=wk_s, in_=wkT.rearrange("(dd p) m -> p dd m", p=128))
            wv_s = const.tile([128, KD, M], F32R)
            nc.sync.dma_start(out=wv_s, in_=wvT.rearrange("(dd p) m -> p dd m", p=128))
            wo_s = const.tile([128, 2, D], F32R)
            nc.sync.dma_start(out=wo_s, in_=woT.rearrange("(kk p) j -> p kk j", p=128))
            bq_s = const.tile([128, 2], F32)
            nc.sync.dma_start(out=bq_s, in_=bq.rearrange("(mt p) -> p mt", p=128))
            bk_s = const.tile([128, 2], F32)
            nc.sync.dma_start(out=bk_s, in_=bk.rearrange("(mt p) -> p mt", p=128))
            bv_row = const.tile([1, M], F32R)
            nc.sync.dma_start(out=bv_row, in_=bv[None, :])

            ones_t = const.tile([1, 128], F32R)
            nc.sync.dma_start(out=ones_t, in_=ones_d[0:1, :])
            ones1_64 = ones_t[:, 0:64]
            ones1_128 = ones_t[:, :]

            qT_s = const.tile([128, 2, T], F32R)
            kT_s = const.tile([128, 2, T], F32R)
            v_s = const.tile([128, NT, NH_LOC, DH + 1], F32R)
            nc.sync.dma_start(
                out=v_s[:, :, :, DH],
                in_=ones_d.rearrange("p (a b) -> p a b", a=NT)[:, :, 0:NH_LOC],
            )
            yT_s = const.tile([128, 2, T], F32R)

            # ---- Phase B: projections ----
            with tc.tile_pool(name="psB", bufs=2, space="PSUM") as psB:
                for w_s, b_s, dst in ((wq_s, bq_s, qT_s), (wk_s, bk_s, kT_s)):
                    for mt in range(2):
                        for c in range(NC):
                            pp = psB.tile([128, 512], F32, tag="proj")
                            for dd in range(KD):
                                nc.tensor.matmul(
                                    pp,
                                    w_s[:, dd, mt * 128:(mt + 1) * 128],
                                    xs[:, dd, c * 512:(c + 1) * 512],
                                    start=(dd == 0), stop=(dd == KD - 1),
                                )
                            nc.vector.tensor_scalar_add(
                                dst[:, mt, c * 512:(c + 1) * 512], pp, b_s[:, mt:mt + 1]
                            )
                for tb in range(NT):
                    pv = psB.tile([128, M], F32, tag="vproj")
                    for dd in range(KD):
                        nc.tensor.matmul(
                            pv,
                            xs[:, dd, tb * 128:(tb + 1) * 128],
                            wv_s[:, dd, :],
                            start=(dd == 0), stop=False,
                        )
                    nc.tensor.matmul(pv, ones1_128, bv_row, start=False, stop=True)
                    nc.vector.tensor_copy(
                        v_s[:, tb, :, 0:DH], pv.rearrange("p (h d) -> p h d", h=NH_LOC)
                    )

            # ---- Phase C: attention per head ----
            with (
                tc.tile_pool(name="psS", bufs=2, space="PSUM") as psS,
                tc.tile_pool(name="psY", bufs=1, space="PSUM") as psY,
                tc.tile_pool(name="psBC", bufs=2, space="PSUM") as psBC,
                tc.tile_pool(name="pch", bufs=3) as pch,
                tc.tile_pool(name="tails", bufs=2) as tails,
            ):
                for h in range(NH_LOC):
                    mt_h, po = h // 2, (h % 2) * 64
                    qT_h = qT_s[po:po + 64, mt_h, :]
                    kT_h = kT_s[po:po + 64, mt_h, :]
                    yt = psY.tile([65, T], F32, tag="yt")
                    for c in range(NC):
                        csl = slice(c * 512, (c + 1) * 512)
                        for j in range(4 * c + 4):
                            st = psS.tile([128, 512], F32, tag="st")
                            nc.tensor.matmul(
                                st, kT_h[:, j * 128:(j + 1) * 128], qT_h[:, csl],
                                start=True, stop=True,
                            )
                            p_ch = pch.tile([128, 512], F32R, tag="p")
                            nc.scalar.activation(p_ch, st, Exp)
                            if j // 4 == c:
                                w = (j % 4 + 1) * 128
                                nc.gpsimd.affine_select(
                                    out=p_ch[:, 0:w], in_=p_ch[:, 0:w],
                                    compare_op=mybir.AluOpType.is_ge, fill=0.0,
                                    base=c * 512 - j * 128,
                                    channel_multiplier=-1, pattern=[[1, w]],
                                )
                            nc.tensor.matmul(
                                yt[:, csl], v_s[:, j, h, :], p_ch,
                                start=(j == 0), stop=(j == 4 * c + 3),
                            )
                        # normalize this chunk: reciprocal of denom row, PE-broadcast, mul
                        den = tails.tile([1, 512], F32, tag="den")
                        nc.vector.tensor_copy(den, yt[64:65, csl])
                        rec = tails.tile([1, 512], F32R, tag="rec")
                        with nc.allow_low_precision(reason="float32r is fp32-width"):
                            nc.vector.reciprocal(rec, den)
                        bc = psBC.tile([64, 512], F32, tag="bc")
                        nc.tensor.matmul(bc, ones1_64, rec, start=True, stop=True)
                        bc_sb = tails.tile([64, 512], F32, tag="bcs")
                        nc.vector.tensor_copy(bc_sb, bc)
                        nc.vector.tensor_mul(
                            yT_s[po:po + 64, mt_h, csl], yt[0:64, csl], bc_sb
                        )

            # ---- Phase D: output projection (partial; host adds bo and reduces) ----
            with (
                tc.tile_pool(name="psD", bufs=2, space="PSUM") as psD,
                tc.tile_pool(name="outs", bufs=3) as outs,
            ):
                out_r = outp.rearrange("(tb p) j -> tb p j", p=128)
                for tb in range(NT):
                    po_t = psD.tile([128, D], F32, tag="oproj")
                    for n in range(2):
                        for kk in range(2):
                            nc.tensor.matmul(
                                po_t[:, n * 512:(n + 1) * 512],
                                yT_s[:, kk, tb * 128:(tb + 1) * 128],
                                wo_s[:, kk, n * 512:(n + 1) * 512],
                                start=(kk == 0), stop=(kk == 1),
                            )
                    o_sb = outs.tile([128, D], F32, tag="out")
                    nc.vector.tensor_copy(o_sb, po_t)
                    nc.sync.dma_start(out=out_r[tb], in_=o_sb)

    nc.compile()
    return nc


_NC = None


def _get_nc():
    global _NC
    if _NC is None:
        _NC = _build()
    return _NC


def kernel(x, Wq, bq, Wk, bk, Wv, bv, Wo, bo, _trace=False):
    x = np.ascontiguousarray(np.asarray(x, dtype=np.float32))
    Wq = np.asarray(Wq, dtype=np.float32)
    Wk = np.asarray(Wk, dtype=np.float32)
    Wv = np.asarray(Wv, dtype=np.float32)
    Wo = np.asarray(Wo, dtype=np.float32)
    bq = np.asarray(bq, dtype=np.float32)
    bk = np.asarray(bk, dtype=np.float32)
    bv = np.asarray(bv, dtype=np.float32)
    bo = np.asarray(bo, dtype=np.float32)

    scale = np.float32(1.0 / np.sqrt(DH))
    ones_d = np.ones((128, 128), dtype=np.float32)
    in_maps = []
    for c in range(8):
        b, roff = c // 4, (c % 4) * M
        in_maps.append({
            "ones_d": ones_d,
            "xT": np.ascontiguousarray(x[b].T),
            "wqT": np.ascontiguousarray((Wq[roff:roff + M] * scale).T),
            "wkT": np.ascontiguousarray(Wk[roff:roff + M].T),
            "wvT": np.ascontiguousarray(Wv[roff:roff + M].T),
            "bq": np.ascontiguousarray(bq[roff:roff + M] * scale),
            "bk": np.ascontiguousarray(bk[roff:roff + M]),
            "bv": np.ascontiguousarray(bv[roff:roff + M]),
            "woT": np.ascontiguousarray(Wo[:, roff:roff + M].T),
        })

    nc = _get_nc()
    res = run_bass_kernel_spmd(nc, in_maps, list(range(8)), trace=_trace)

    out = np.empty((B, T, D), dtype=np.float32)
    for b in range(B):
        acc = np.zeros((T, D), dtype=np.float64)
        for c in range(4 * b, 4 * b + 4):
            acc += res.results[c]["outp"]
        out[b] = (acc + bo.astype(np.float64)).astype(np.float32)
    if _trace:
        kernel.last_results = res
    return out


# revision 31
# speedup vs baseline: 1.3177x; 1.3177x over previous
"""Causal self-attention (B=2, T=2048, D=1024, H=16, Dh=64) on 8 Trainium2 cores.

Sharding: (batch, head-group) — core c handles batch c//4 and heads 4*(c%4)..+4.
Each core computes Q/K/V projections for its 4 heads, causal attention, and a
partial output projection (its head-columns of Wo); the host sums the 4 partial
outputs per batch and adds bo.

Per-core layouts (all matmuls in float32r = fp32 bits on the fast PE path):
  xT   [D=1024, T=2048]   x[b] transposed (d on partitions)
  qT/kT [256, T]          head-major projections, d-of-head on partitions
  v    [T, 4, 65]         natural orientation + ones column (softmax denom trick)
  S^T  [j, i] chunks      scores transposed: exp() then causal zeroing, no max
                          subtraction (|scores| <= ~10, fp32-safe)
  y'^T [65, T] psum       unnormalized attention out + denominator row via the
                          ones column of V; normalized by PE-broadcast reciprocal
"""
import numpy as np

import concourse.bacc as bacc
import concourse.bass as bass
import concourse.mybir as mybir
import concourse.tile as tile
from concourse.bass_utils import run_bass_kernel_spmd

F32 = mybir.dt.float32
F32R = mybir.dt.float32r

B, T, D = 2, 2048, 1024
NH_LOC, DH = 4, 64          # heads per core, head dim
M = NH_LOC * DH             # 256 local qkv dims
KD = D // 128               # 8 contraction chunks
NT = T // 128               # 16 t-blocks
NC = T // 512               # 4 512-chunks
Exp = mybir.ActivationFunctionType.Exp


def _build():
    nc = bacc.Bacc("TRN2", target_bir_lowering=False, debug=False, num_devices=8)

    xT = nc.dram_tensor("xT", [D, T], F32R, kind="ExternalInput")
    wqT = nc.dram_tensor("wqT", [D, M], F32R, kind="ExternalInput")
    wkT = nc.dram_tensor("wkT", [D, M], F32R, kind="ExternalInput")
    wvT = nc.dram_tensor("wvT", [D, M], F32R, kind="ExternalInput")
    bq = nc.dram_tensor("bq", [M], F32, kind="ExternalInput")
    bk = nc.dram_tensor("bk", [M], F32, kind="ExternalInput")
    bv = nc.dram_tensor("bv", [M], F32R, kind="ExternalInput")
    woT = nc.dram_tensor("woT", [M, D], F32R, kind="ExternalInput")
    ones_d = nc.dram_tensor("ones_d", [128, 128], F32R, kind="ExternalInput")
    outp = nc.dram_tensor("outp", [T, D], mybir.dt.bfloat16, kind="ExternalOutput")

    with tile.TileContext(nc) as tc:
        with (
            tc.tile_pool(name="const", bufs=1) as const,
            tc.tile_pool(name="xpool", bufs=1) as xpool,
        ):
            xs = xpool.tile([128, KD, T], F32R)
            xr = xT.rearrange("(dd p) t -> p dd t", p=128)
            wq_s = const.tile([128, KD, M], F32R)
            nc.sync.dma_start(out=wq_s, in_=wqT.rearrange("(dd p) m -> p dd m", p=128))
            wk_s = const.tile([128, KD, M], F32R)
            nc.sync.dma_start(out=wk_s, in_=wkT.rearrange("(dd p) m -> p dd m", p=128))
            wv_s = const.tile([128, KD, M], F32R)
            nc.sync.dma_start(out=wv_s, in_=wvT.rearrange("(dd p) m -> p dd m", p=128))
            bq_s = const.tile([128, 2], F32)
            nc.sync.dma_start(out=bq_s, in_=bq.rearrange("(mt p) -> p mt", p=128))
            bk_s = const.tile([128, 2], F32)
            nc.sync.dma_start(out=bk_s, in_=bk.rearrange("(mt p) -> p mt", p=128))
            bv_row = const.tile([1, M], F32R)
            nc.sync.dma_start(out=bv_row, in_=bv[None, :])
            ones_t = const.tile([1, 128], F32R)
            nc.sync.dma_start(out=ones_t, in_=ones_d[0:1, :])
            ones1_64 = ones_t[:, 0:64]
            ones1_128 = ones_t[:, :]
            v_s = const.tile([128, NT, NH_LOC, DH + 1], F32R)
            nc.sync.dma_start(
                out=v_s[:, :, :, DH],
                in_=ones_d.rearrange("p (a b) -> p a b", a=NT)[:, :, 0:NH_LOC],
            )
            for c in range(NC):
                nc.sync.dma_start(
                    out=xs[:, :, c * 512:(c + 1) * 512], in_=xr[:, :, c * 512:(c + 1) * 512]
                )
            wo_s = const.tile([128, 2, D], F32R)
            nc.sync.dma_start(out=wo_s, in_=woT.rearrange("(kk p) j -> p kk j", p=128))

            qT_s = const.tile([128, 2, T], F32R)
            kT_s = const.tile([128, 2, T], F32R)
            yT_s = const.tile([128, 2, T], F32R)

            # ---- Phases B and C, interleaved ----
            # B: projections, per x-chunk so PE keeps up with the streaming x DMA.
            # Heads 0/1 Q/K (m-tile 0) + V projected first; heads 2/3 Q/K
            # (m-tile 1) are emitted between C(h1) and C(h2) so the scheduler
            # can hide them under the ACT-bound attention of heads 0/1.
            # PSUM banks: proj 1 + st 2x2 + yt 1x3 = 8.
            with (
                tc.tile_pool(name="psB", bufs=1, space="PSUM") as psB,
                tc.tile_pool(name="psS", bufs=2, space="PSUM") as psS,
                tc.tile_pool(name="psY", bufs=3, space="PSUM") as psY,
                tc.tile_pool(name="pch", bufs=3) as pch,
                tc.tile_pool(name="tails", bufs=3) as tails,
            ):
                def proj_qk(mt, c):
                    for w_s, b_s, dst in ((wq_s, bq_s, qT_s), (wk_s, bk_s, kT_s)):
                        pp = psB.tile([128, 512], F32, tag="proj")
                        for dd in range(KD):
                            nc.tensor.matmul(
                                pp,
                                w_s[:, dd, mt * 128:(mt + 1) * 128],
                                xs[:, dd, c * 512:(c + 1) * 512],
                                start=(dd == 0), stop=(dd == KD - 1),
                            )
                        nc.vector.tensor_scalar_add(
                            dst[:, mt, c * 512:(c + 1) * 512], pp, b_s[:, mt:mt + 1]
                        )

                def proj_v(c):
                    for tb in range(4 * c, 4 * c + 4):
                        pv = psB.tile([128, M], F32, tag="proj")
                        for dd in range(KD):
                            nc.tensor.matmul(
                                pv,
                                xs[:, dd, tb * 128:(tb + 1) * 128],
                                wv_s[:, dd, :],
                                start=(dd == 0), stop=False,
                            )
                        nc.tensor.matmul(pv, ones1_128, bv_row, start=False, stop=True)
                        nc.vector.tensor_copy(
                            v_s[:, tb, :, 0:DH], pv.rearrange("p (h d) -> p h d", h=NH_LOC)
                        )

                def tail(h, yt, c):
                    # normalize chunk c: reciprocal of denom row (65th V column),
                    # PE-broadcast into yt's free partitions, DVE multiply
                    mt_h, po = h // 2, (h % 2) * 64
                    rec = tails.tile([1, 512], F32R, tag="rec")
                    with nc.allow_low_precision(reason="float32r is fp32-width"):
                        nc.vector.reciprocal(rec, yt[64:65, :])
                    bc = psY.tile([64, 512], F32, tag="yt", name=f"bc_{h}_{c}")
                    nc.tensor.matmul(bc, ones1_64, rec, start=True, stop=True)
                    bc_sb = tails.tile([64, 512], F32, tag="bcs")
                    nc.vector.tensor_copy(bc_sb, bc)
                    nc.vector.tensor_mul(
                        yT_s[po:po + 64, mt_h, c * 512:(c + 1) * 512],
                        yt[0:64, :], bc_sb,
                    )

                def attn_head(h):
                    # S^T strips of width 1024 (2 PSUM banks); exp/QK/AV trimmed
                    # to the causal-valid region, left-padded to keep fp32r
                    # moving width >= 256.
                    qT_h = qT_s[(h % 2) * 64:(h % 2) * 64 + 64, h // 2, :]
                    kT_h = kT_s[(h % 2) * 64:(h % 2) * 64 + 64, h // 2, :]
                    for c2 in range(2):
                        base = c2 * 1024
                        yts = {c: psY.tile([128, 512], F32, tag="yt", name=f"yt_{h}_{c}")
                               for c in (2 * c2, 2 * c2 + 1)}
                        for j in range(8 * c2 + 8):
                            diag = j * 128 >= base
                            d_rel = j * 128 - base  # valid cols start (if diag)
                            if diag:
                                qk0 = min(d_rel, 768) if d_rel >= 512 else min(d_rel, 256)
                            else:
                                qk0 = 0
                            st = psS.tile([128, 1024], F32, tag="st")
                            p_ch = pch.tile([128, 1024], F32R, tag="p")
                            bounds = [qk0, 512, 1024] if qk0 < 512 else [qk0, 1024]
                            for lo, hi in zip(bounds[:-1], bounds[1:]):
                                nc.tensor.matmul(
                                    st[:, lo:hi],
                                    kT_h[:, j * 128:(j + 1) * 128],
                                    qT_h[:, base + lo:base + hi],
                                    start=True, stop=True,
                                )
                            nc.scalar.activation(p_ch[:, qk0:1024], st[:, qk0:1024], Exp)
                            if diag:
                                # zero left-of-diagonal + upper triangle in one
                                # select: keep iff global_i >= global_j
                                w = d_rel + 128 - qk0
                                nc.gpsimd.affine_select(
                                    out=p_ch[:, qk0:qk0 + w], in_=p_ch[:, qk0:qk0 + w],
                                    compare_op=mybir.AluOpType.is_ge, fill=0.0,
                                    base=qk0 - d_rel,
                                    channel_multiplier=-1, pattern=[[1, w]],
                                )
                            for c in (2 * c2, 2 * c2 + 1):
                                r0 = (c - 2 * c2) * 512
                                if diag and d_rel >= r0 + 512:
                                    continue  # sub-chunk fully masked
                                av0 = max(r0, min(d_rel, r0 + 256)) if diag else r0
                                nc.tensor.matmul(
                                    yts[c][0:65, (av0 - r0):512],
                                    v_s[:, j, h, :],
                                    p_ch[:, av0:r0 + 512],
                                    start=(j == 0), stop=(j == 4 * c + 3),
                                )
                                if j == 4 * c + 3:
                                    tail(h, yts[c], c)

                for c in range(NC):
                    proj_qk(0, c)
                    proj_v(c)
                attn_head(0)
                attn_head(1)
                for c in range(NC):
                    proj_qk(1, c)
                attn_head(2)
                attn_head(3)

            # ---- Phase D: output projection (partial; host adds bo and reduces) ----
            with (
                tc.tile_pool(name="psD", bufs=3, space="PSUM") as psD,
                tc.tile_pool(name="outs", bufs=3) as outs,
            ):
                out_r = outp.rearrange("(tb p) j -> tb p j", p=128)
                for tb in range(NT):
                    po_t = psD.tile([128, D], F32, tag="oproj")
                    for n in range(2):
                        for kk in range(2):
                            nc.tensor.matmul(
                                po_t[:, n * 512:(n + 1) * 512],
                                yT_s[:, kk, tb * 128:(tb + 1) * 128],
                                wo_s[:, kk, n * 512:(n + 1) * 512],
                                start=(kk == 0), stop=(kk == 1),
                            )
                    o_sb = outs.tile([128, D], mybir.dt.bfloat16, tag="out")
                    with nc.allow_low_precision(reason="partial out; host sums in f32"):
                        if tb % 2 == 0:
                            nc.vector.tensor_copy(o_sb, po_t)
                        else:
                            nc.scalar.copy(o_sb, po_t)
                    nc.sync.dma_start(out=out_r[tb], in_=o_sb)

    nc.compile()
    return nc


_NC = None


def _get_nc():
    global _NC
    if _NC is None:
        _NC = _build()
    return _NC


def kernel(x, Wq, bq, Wk, bk, Wv, bv, Wo, bo, _trace=False):
    x = np.ascontiguousarray(np.asarray(x, dtype=np.float32))
    Wq = np.asarray(Wq, dtype=np.float32)
    Wk = np.asarray(Wk, dtype=np.float32)
    Wv = np.asarray(Wv, dtype=np.float32)
    Wo = np.asarray(Wo, dtype=np.float32)
    bq = np.asarray(bq, dtype=np.float32)
    bk = np.asarray(bk, dtype=np.float32)
    bv = np.asarray(bv, dtype=np.float32)
    bo = np.asarray(bo, dtype=np.float32)

    scale = np.float32(1.0 / np.sqrt(DH))
    ones_d = np.ones((128, 128), dtype=np.float32)
    in_maps = []
    for c in range(8):
        b, roff = c // 4, (c % 4) * M
        in_maps.append({
            "ones_d": ones_d,
            "xT": np.ascontiguousarray(x[b].T),
            "wqT": np.ascontiguousarray((Wq[roff:roff + M] * scale).T),
            "wkT": np.ascontiguousarray(Wk[roff:roff + M].T),
            "wvT": np.ascontiguousarray(Wv[roff:roff + M].T),
            "bq": np.ascontiguousarray(bq[roff:roff + M] * scale),
            "bk": np.ascontiguousarray(bk[roff:roff + M]),
            "bv": np.ascontiguousarray(bv[roff:roff + M]),
            "woT": np.ascontiguousarray(Wo[:, roff:roff + M].T),
        })

    nc = _get_nc()
    res = run_bass_kernel_spmd(nc, in_maps, list(range(8)), trace=_trace)

    out = np.empty((B, T, D), dtype=np.float32)
    for b in range(B):
        acc = np.zeros((T, D), dtype=np.float64)
        for c in range(4 * b, 4 * b + 4):
            acc += res.results[c]["outp"]
        out[b] = (acc + bo.astype(np.float64)).astype(np.float32)
    if _trace:
        kernel.last_results = res
    return out


# revision 42
# speedup vs baseline: 24752.9814x; 18785.0082x over previous
"""Causal self-attention (B=2, T=2048, D=1024, H=16, Dh=64) on 8 Trainium2 cores.

Sharding: (batch, head-group) — core c handles batch c//4 and heads 4*(c%4)..+4.
Each core computes Q/K/V projections for its 4 heads, causal attention, and a
partial output projection (its head-columns of Wo); the host sums the 4 partial
outputs per batch and adds bo.

Per-core layouts (all matmuls in float32r = fp32 bits on the fast PE path):
  xT   [D=1024, T=2048]   x[b] transposed (d on partitions)
  qT/kT [256, T]          head-major projections, d-of-head on partitions
  v    [T, 4, 65]         natural orientation + ones column (softmax denom trick)
  S^T  [j, i] chunks      scores transposed: exp() then causal zeroing, no max
                          subtraction (|scores| <= ~10, fp32-safe)
  y'^T [65, T] psum       unnormalized attention out + denominator row via the
                          ones column of V; normalized by PE-broadcast reciprocal
"""
import numpy as np

import concourse.bacc as bacc
import concourse.bass as bass
import concourse.mybir as mybir
import concourse.tile as tile
from concourse.bass_utils import run_bass_kernel_spmd

F32 = mybir.dt.float32
F32R = mybir.dt.float32r

B, T, D = 2, 2048, 1024
NH_LOC, DH = 4, 64          # heads per core, head dim
M = NH_LOC * DH             # 256 local qkv dims
KD = D // 128               # 8 contraction chunks
NT = T // 128               # 16 t-blocks
NC = T // 512               # 4 512-chunks
Exp = mybir.ActivationFunctionType.Exp


def _build_lite():
    """Signature-identical DMA-only kernel, for overhead-delta timing."""
    nc = bacc.Bacc("TRN2", target_bir_lowering=False, debug=False, num_devices=8)
    xT = nc.dram_tensor("xT", [D, T], F32R, kind="ExternalInput")
    wqT = nc.dram_tensor("wqT", [D, M], F32R, kind="ExternalInput")
    wkT = nc.dram_tensor("wkT", [D, M], F32R, kind="ExternalInput")
    wvT = nc.dram_tensor("wvT", [D, M], F32R, kind="ExternalInput")
    bq = nc.dram_tensor("bq", [M], F32, kind="ExternalInput")
    bk = nc.dram_tensor("bk", [M], F32, kind="ExternalInput")
    bv = nc.dram_tensor("bv", [M], F32R, kind="ExternalInput")
    woT = nc.dram_tensor("woT", [M, D], F32R, kind="ExternalInput")
    ones_d = nc.dram_tensor("ones_d", [128, 128], F32R, kind="ExternalInput")
    outp = nc.dram_tensor("outp", [T, D], mybir.dt.bfloat16, kind="ExternalOutput")
    with tile.TileContext(nc) as tc:
        with tc.tile_pool(name="const", bufs=1) as const, tc.tile_pool(name="lo", bufs=2) as lo:
            xs = const.tile([128, KD, T], F32R)
            nc.sync.dma_start(out=xs, in_=xT.rearrange("(dd p) t -> p dd t", p=128))
            for w, shp in ((wqT, [128, KD, M]), (wkT, [128, KD, M]), (wvT, [128, KD, M])):
                ws = const.tile(shp, F32R, name=f"w_{w.name}")
                nc.sync.dma_start(out=ws, in_=w.rearrange("(dd p) m -> p dd m", p=128))
            wos = const.tile([128, 2, D], F32R)
            nc.sync.dma_start(out=wos, in_=woT.rearrange("(kk p) j -> p kk j", p=128))
            for bt in (bq, bk, bv):
                bs = const.tile([128, 2], bt.dtype, name=f"b_{bt.name}")
                nc.sync.dma_start(out=bs, in_=bt.rearrange("(mt p) -> p mt", p=128))
            os_ = const.tile([1, 128], F32R)
            nc.sync.dma_start(out=os_, in_=ones_d[0:1, :])
            out_r = outp.rearrange("(tb p) j -> tb p j", p=128)
            for tb in range(NT):
                z = lo.tile([128, D], mybir.dt.bfloat16, tag="z")
                nc.vector.memset(z, 0.0)
                nc.sync.dma_start(out=out_r[tb], in_=z)
    nc.compile()
    return nc


def _build():
    nc = bacc.Bacc("TRN2", target_bir_lowering=False, debug=False, num_devices=8)

    xT = nc.dram_tensor("xT", [D, T], F32R, kind="ExternalInput")
    wqT = nc.dram_tensor("wqT", [D, M], F32R, kind="ExternalInput")
    wkT = nc.dram_tensor("wkT", [D, M], F32R, kind="ExternalInput")
    wvT = nc.dram_tensor("wvT", [D, M], F32R, kind="ExternalInput")
    bq = nc.dram_tensor("bq", [M], F32, kind="ExternalInput")
    bk = nc.dram_tensor("bk", [M], F32, kind="ExternalInput")
    bv = nc.dram_tensor("bv", [M], F32R, kind="ExternalInput")
    woT = nc.dram_tensor("woT", [M, D], F32R, kind="ExternalInput")
    ones_d = nc.dram_tensor("ones_d", [128, 128], F32R, kind="ExternalInput")
    outp = nc.dram_tensor("outp", [T, D], mybir.dt.bfloat16, kind="ExternalOutput")

    with tile.TileContext(nc) as tc:
        with (
            tc.tile_pool(name="const", bufs=1) as const,
            tc.tile_pool(name="xpool", bufs=1) as xpool,
        ):
            xs = xpool.tile([128, KD, T], F32R)
            xr = xT.rearrange("(dd p) t -> p dd t", p=128)
            wq_s = const.tile([128, KD, M], F32R)
            nc.sync.dma_start(out=wq_s, in_=wqT.rearrange("(dd p) m -> p dd m", p=128))
            wk_s = const.tile([128, KD, M], F32R)
            nc.sync.dma_start(out=wk_s, in_=wkT.rearrange("(dd p) m -> p dd m", p=128))
            wv_s = const.tile([128, KD, M], F32R)
            nc.sync.dma_start(out=wv_s, in_=wvT.rearrange("(dd p) m -> p dd m", p=128))
            bq_s = const.tile([128, 2], F32)
            nc.sync.dma_start(out=bq_s, in_=bq.rearrange("(mt p) -> p mt", p=128))
            bk_s = const.tile([128, 2], F32)
            nc.sync.dma_start(out=bk_s, in_=bk.rearrange("(mt p) -> p mt", p=128))
            bv_row = const.tile([1, M], F32R)
            nc.sync.dma_start(out=bv_row, in_=bv[None, :])
            ones_t = const.tile([1, 128], F32R)
            nc.sync.dma_start(out=ones_t, in_=ones_d[0:1, :])
            ones1_64 = ones_t[:, 0:64]
            ones1_128 = ones_t[:, :]
            v_s = const.tile([128, NT, NH_LOC, DH + 1], F32R)
            nc.sync.dma_start(
                out=v_s[:, :, :, DH],
                in_=ones_d.rearrange("p (a b) -> p a b", a=NT)[:, :, 0:NH_LOC],
            )
            for c in range(NC):
                nc.sync.dma_start(
                    out=xs[:, :, c * 512:(c + 1) * 512], in_=xr[:, :, c * 512:(c + 1) * 512]
                )
            wo_s = const.tile([128, 2, D], F32R)
            nc.sync.dma_start(out=wo_s, in_=woT.rearrange("(kk p) j -> p kk j", p=128))

            qT_s = const.tile([128, 2, T], F32R)
            kT_s = const.tile([128, 2, T], F32R)
            yT_s = const.tile([128, 2, T], F32R)

            # ---- Phases B and C, interleaved ----
            # B: projections, per x-chunk so PE keeps up with the streaming x DMA.
            # Heads 0/1 Q/K (m-tile 0) + V projected first; heads 2/3 Q/K
            # (m-tile 1) are emitted between C(h1) and C(h2) so the scheduler
            # can hide them under the ACT-bound attention of heads 0/1.
            # PSUM banks: proj 1 + st 2x2 + yt 1x3 = 8.
            with (
                tc.tile_pool(name="psB", bufs=1, space="PSUM") as psB,
                tc.tile_pool(name="psS", bufs=2, space="PSUM") as psS,
                tc.tile_pool(name="psY", bufs=3, space="PSUM") as psY,
                tc.tile_pool(name="pch", bufs=3) as pch,
                tc.tile_pool(name="tails", bufs=3) as tails,
            ):
                def proj_qk(mt, c):
                    for w_s, b_s, dst in ((wq_s, bq_s, qT_s), (wk_s, bk_s, kT_s)):
                        pp = psB.tile([128, 512], F32, tag="proj")
                        for dd in range(KD):
                            nc.tensor.matmul(
                                pp,
                                w_s[:, dd, mt * 128:(mt + 1) * 128],
                                xs[:, dd, c * 512:(c + 1) * 512],
                                start=(dd == 0), stop=(dd == KD - 1),
                            )
                        nc.vector.tensor_scalar_add(
                            dst[:, mt, c * 512:(c + 1) * 512], pp, b_s[:, mt:mt + 1]
                        )

                def proj_v(c):
                    for tb in range(4 * c, 4 * c + 4):
                        pv = psB.tile([128, M], F32, tag="proj")
                        for dd in range(KD):
                            nc.tensor.matmul(
                                pv,
                                xs[:, dd, tb * 128:(tb + 1) * 128],
                                wv_s[:, dd, :],
                                start=(dd == 0), stop=False,
                            )
                        nc.tensor.matmul(pv, ones1_128, bv_row, start=False, stop=True)
                        nc.vector.tensor_copy(
                            v_s[:, tb, :, 0:DH], pv.rearrange("p (h d) -> p h d", h=NH_LOC)
                        )

                def tail(h, yt, c):
                    # normalize chunk c: reciprocal of denom row (65th V column),
                    # PE-broadcast into yt's free partitions, DVE multiply
                    mt_h, po = h // 2, (h % 2) * 64
                    rec = tails.tile([1, 512], F32R, tag="rec")
                    with nc.allow_low_precision(reason="float32r is fp32-width"):
                        nc.vector.reciprocal(rec, yt[64:65, :])
                    bc = psY.tile([64, 512], F32, tag="yt", name=f"bc_{h}_{c}")
                    nc.tensor.matmul(bc, ones1_64, rec, start=True, stop=True)
                    bc_sb = tails.tile([64, 512], F32, tag="bcs")
                    nc.vector.tensor_copy(bc_sb, bc)
                    nc.vector.tensor_mul(
                        yT_s[po:po + 64, mt_h, c * 512:(c + 1) * 512],
                        yt[0:64, :], bc_sb,
                    )

                def attn_head(h, interleave=None):
                    # S^T strips of width 1024 (2 PSUM banks); exp/QK/AV trimmed
                    # to the causal-valid region, left-padded to keep fp32r
                    # moving width >= 256.
                    qT_h = qT_s[(h % 2) * 64:(h % 2) * 64 + 64, h // 2, :]
                    kT_h = kT_s[(h % 2) * 64:(h % 2) * 64 + 64, h // 2, :]
                    for c2 in range(2):
                        if interleave is not None:
                            interleave(h * 2 + c2)
                        base = c2 * 1024
                        yts = {c: psY.tile([128, 512], F32, tag="yt", name=f"yt_{h}_{c}")
                               for c in (2 * c2, 2 * c2 + 1)}
                        for j in range(8 * c2 + 8):
                            diag = j * 128 >= base
                            d_rel = j * 128 - base  # valid cols start (if diag)
                            if diag:
                                qk0 = min(d_rel, 768) if d_rel >= 512 else min(d_rel, 256)
                            else:
                                qk0 = 0
                            st = psS.tile([128, 1024], F32, tag="st")
                            p_ch = pch.tile([128, 1024], F32R, tag="p")
                            bounds = [qk0, 512, 1024] if qk0 < 512 else [qk0, 1024]
                            for lo, hi in zip(bounds[:-1], bounds[1:]):
                                nc.tensor.matmul(
                                    st[:, lo:hi],
                                    kT_h[:, j * 128:(j + 1) * 128],
                                    qT_h[:, base + lo:base + hi],
                                    start=True, stop=True,
                                )
                            nc.scalar.activation(p_ch[:, qk0:1024], st[:, qk0:1024], Exp)
                            if diag:
                                # zero left-of-diagonal + upper triangle in one
                                # select: keep iff global_i >= global_j
                                w = d_rel + 128 - qk0
                                nc.gpsimd.affine_select(
                                    out=p_ch[:, qk0:qk0 + w], in_=p_ch[:, qk0:qk0 + w],
                                    compare_op=mybir.AluOpType.is_ge, fill=0.0,
                                    base=qk0 - d_rel,
                                    channel_multiplier=-1, pattern=[[1, w]],
                                )
                            for c in (2 * c2, 2 * c2 + 1):
                                r0 = (c - 2 * c2) * 512
                                if diag and d_rel >= r0 + 512:
                                    continue  # sub-chunk fully masked
                                av0 = max(r0, min(d_rel, r0 + 256)) if diag else r0
                                nc.tensor.matmul(
                                    yts[c][0:65, (av0 - r0):512],
                                    v_s[:, j, h, :],
                                    p_ch[:, av0:r0 + 512],
                                    start=(j == 0), stop=(j == 4 * c + 3),
                                )
                                if j == 4 * c + 3:
                                    tail(h, yts[c], c)

                for c in range(NC):
                    proj_qk(0, c)
                    proj_v(c)
                attn_head(0)
                attn_head(1)
                for c in range(NC):
                    proj_qk(1, c)
                attn_head(2)
                attn_head(3)

            # ---- Phase D: output projection (partial; host adds bo and reduces) ----
            with (
                tc.tile_pool(name="psD", bufs=3, space="PSUM") as psD,
                tc.tile_pool(name="outs", bufs=3) as outs,
            ):
                out_r = outp.rearrange("(tb p) j -> tb p j", p=128)
                for tb in range(NT):
                    po_t = psD.tile([128, D], F32, tag="oproj")
                    for n in range(2):
                        for kk in range(2):
                            nc.tensor.matmul(
                                po_t[:, n * 512:(n + 1) * 512],
                                yT_s[:, kk, tb * 128:(tb + 1) * 128],
                                wo_s[:, kk, n * 512:(n + 1) * 512],
                                start=(kk == 0), stop=(kk == 1),
                            )
                    o_sb = outs.tile([128, D], mybir.dt.bfloat16, tag="out")
                    with nc.allow_low_precision(reason="partial out; host sums in f32"):
                        if tb % 2 == 0:
                            nc.vector.tensor_copy(o_sb, po_t)
                        else:
                            nc.scalar.copy(o_sb, po_t)
                    nc.sync.dma_start(out=out_r[tb], in_=o_sb)

    nc.compile()
    return nc


_NC = None


def _get_nc():
    global _NC
    if _NC is None:
        _NC = _build()
    return _NC


def kernel(x, Wq, bq, Wk, bk, Wv, bv, Wo, bo, _trace=False):
    x = np.ascontiguousarray(np.asarray(x, dtype=np.float32))
    Wq = np.asarray(Wq, dtype=np.float32)
    Wk = np.asarray(Wk, dtype=np.float32)
    Wv = np.asarray(Wv, dtype=np.float32)
    Wo = np.asarray(Wo, dtype=np.float32)
    bq = np.asarray(bq, dtype=np.float32)
    bk = np.asarray(bk, dtype=np.float32)
    bv = np.asarray(bv, dtype=np.float32)
    bo = np.asarray(bo, dtype=np.float32)

    scale = np.float32(1.0 / np.sqrt(DH))
    ones_d = np.ones((128, 128), dtype=np.float32)
    in_maps = []
    for c in range(8):
        b, roff = c // 4, (c % 4) * M
        in_maps.append({
            "ones_d": ones_d,
            "xT": np.ascontiguousarray(x[b].T),
            "wqT": np.ascontiguousarray((Wq[roff:roff + M] * scale).T),
            "wkT": np.ascontiguousarray(Wk[roff:roff + M].T),
            "wvT": np.ascontiguousarray(Wv[roff:roff + M].T),
            "bq": np.ascontiguousarray(bq[roff:roff + M] * scale),
            "bk": np.ascontiguousarray(bk[roff:roff + M]),
            "bv": np.ascontiguousarray(bv[roff:roff + M]),
            "woT": np.ascontiguousarray(Wo[:, roff:roff + M].T),
        })

    nc = _get_nc()
    res = run_bass_kernel_spmd(nc, in_maps, list(range(8)), trace=_trace)

    out = np.empty((B, T, D), dtype=np.float32)
    for b in range(B):
        acc = np.zeros((T, D), dtype=np.float64)
        for c in range(4 * b, 4 * b + 4):
            acc += res.results[c]["outp"]
        out[b] = (acc + bo.astype(np.float64)).astype(np.float32)
    if _trace:
        kernel.last_results = res
    return out
